# revision 36
# baseline (speedup 1.0000x reference)
"""Chamfer distance loss on 8 Trainium2 NeuronCores.

Problem: prediction [4, 8192, 3], target [4, 8192, 3] (f32).
  d2[b,n,m] = ||pred[b,n] - tgt[b,m]||^2  (clamped at 0)
  out = mean_{b,n} min_m d2  +  mean_{b,m} min_n d2     (scalar f32)

Sharding: 8 cores = 4 batches x 2 halves of the N axis. Each core computes
its 4096 x 8192 block of the distance matrix flash-style (never
materialized in DRAM):

  - d2 tiles are produced by a single K=24 bf16 matmul per [128,512] tile
    using the augmented-vector trick  d2 = 1*y2 + x2*1 + sum_i (-2 x_i)*y_i
    with every f32 factor split into 3 bf16 limbs (hi/mid/lo) so products
    are exact to ~2^-26 while the PE runs at full bf16 speed (fp32 matmul
    is 4x slower on the PE).
  - ScalarE drains PSUM -> SBUF converting to f16 (values, not operands,
    so rounding is relative: ~2^-11).
  - VectorE keeps a running per-column min (elementwise f16 min, 2x mode)
    and computes each row min with one tensor_tensor_scan(min,min) over the
    two row halves (the last scan column is the full row min). The fused
    reduce ops tensor_tensor_reduce / tensor_scalar+accum_out crash the HW
    exec unit in this environment; the scan and plain tensor_tensor /
    tensor_reduce are HW-verified here.
  - Column mins are folded over the partition axis on-device (PE
    transpose + DVE reduce) so outputs are tiny (axon tunnel is slow).

The paired 24-row operand matrices are host-assembled and shipped as one
flat bf16 array per core (the axon tunnel charges ~100ms per staged
array, so fewer/larger arrays win).

Host combines: per-batch row mins (exact concat) + per-column partial
mins (elementwise min of the two half-N cores), relu, means.
"""

import sys

if "/opt/trn_rl_repo" not in sys.path:
    sys.path.insert(0, "/opt/trn_rl_repo")

import numpy as np
import ml_dtypes


def _install_neff_cache():
    """Cache compiled NEFFs on disk keyed by BIR hash.

    The stock bass_exec path recompiles (~100s of walrus) in every fresh
    process; the program here is deterministic, so a byte-keyed cache is
    safe and makes repeat runs start in seconds.
    """
    import hashlib
    import os
    import shutil

    from concourse import bass2jax as _b2j
    from concourse import bass_utils as _bu

    if getattr(_bu, "_chamfer_neff_cache", False):
        return
    orig = _bu.compile_bir_kernel

    def _key(bir_json):
        # ant_debug blocks embed the kernel.py path/tracebacks, which vary
        # with the directory the module is loaded from — strip before hashing
        try:
            import orjson

            def strip(o):
                if isinstance(o, dict):
                    return {k: strip(v) for k, v in o.items() if k != "ant_debug"}
                if isinstance(o, list):
                    return [strip(v) for v in o]
                return o

            norm = orjson.dumps(strip(orjson.loads(bir_json)))
        except Exception:
            norm = bir_json
        return hashlib.sha256(norm).hexdigest()

    def cached(bir_json, tmpdir, neff_name="file.neff"):
        key = _key(bir_json)
        cdir = os.environ.get("CHAMFER_NEFF_CACHE", "/tmp/chamfer_neff_cache")
        cpath = os.path.join(cdir, key + ".neff")
        out = os.path.join(tmpdir, neff_name)
        try:
            if os.path.exists(cpath):
                shutil.copyfile(cpath, out)
                return out
        except OSError:
            pass
        p = orig(bir_json, tmpdir, neff_name)
        try:
            os.makedirs(cdir, exist_ok=True)
            tmp = cpath + f".tmp{os.getpid()}"
            shutil.copyfile(p, tmp)
            os.replace(tmp, cpath)
        except OSError:
            pass
        return p

    _bu.compile_bir_kernel = cached
    _b2j.compile_bir_kernel = cached
    _bu._chamfer_neff_cache = True


_install_neff_cache()

B, N, M, D = 4, 8192, 8192, 3
N_CORES = 8
NH = N // 2          # rows per core (4096)
P = 128              # partitions
NT = NH // P         # n-tiles per core (32)
K = 24               # contraction rows of the split-bf16 augmented matmul
BIG = 60000.0        # > max possible d2 (~80), fits in f16
XY_LEN = K * NH + K * M  # paired x rows then paired y rows, flat

import os as _os

Z_BUFS = int(_os.environ.get("CHAMFER_Z_BUFS", "2"))
FOLD_STOP = int(_os.environ.get("CHAMFER_FOLD_STOP", "128"))
ROWMIN_MODE = _os.environ.get("CHAMFER_ROWMIN", "scan")  # fold | scan

# Pairing of the 24 product rows: (x source, y source) where sources index
# the 12 "unique" limb rows per side, or "ones" for the constant row.
# x-unique rows: [x2_0, x2_1, x2_2, a00,a01,a02, a10,a11,a12, a20,a21,a22]
# y-unique rows: [y2_0, y2_1, y2_2, b00,b01,b02, b10,b11,b12, b20,b21,b22]
# where a_i* = limbs of -2*x_i and b_i* = limbs of y_i.
PAIRS = (
    [("ones", 0), ("ones", 1), ("ones", 2), (0, "ones"), (1, "ones"), (2, "ones")]
    + [
        (3 + 3 * i + dx, 3 + 3 * i + dy)
        for i in range(3)
        for dx, dy in ((0, 0), (0, 1), (1, 0), (0, 2), (2, 0), (1, 1))
    ]
)
assert len(PAIRS) == K

# Set by test.py.
TRACE = False
LAST_RESULTS = None

_PROGRAM = None


def _build_program():
    from concourse import bacc, tile
    import concourse.mybir as mybir

    f32 = mybir.dt.float32
    f16 = mybir.dt.float16
    bf16 = mybir.dt.bfloat16

    nc = bacc.Bacc(
        "TRN2",
        target_bir_lowering=False,
        debug=False,
        enable_asserts=False,
    )

    xy_d = nc.dram_tensor("xy", [XY_LEN], bf16, kind="ExternalInput").ap()
    rowmin_d = nc.dram_tensor("rowmin", [P, NT], f32, kind="ExternalOutput").ap()
    # colmin[q, k] = min_p over partitions of column m = 128*k + q
    colmin_d = nc.dram_tensor("colmin", [P, M // P], f32, kind="ExternalOutput").ap()

    xh_d = xy_d[0 : K * NH].rearrange("(k n) -> k n", k=K)
    yh_d = xy_d[K * NH :].rearrange("(k n) -> k n", k=K)

    with tile.TileContext(nc) as tc:
        from contextlib import ExitStack

        with ExitStack() as ctx:
            const_pool = ctx.enter_context(tc.tile_pool(name="const", bufs=1))
            z_pool = ctx.enter_context(tc.tile_pool(name="z", bufs=Z_BUFS))
            psum_pool = ctx.enter_context(
                tc.tile_pool(name="psum", bufs=2, space="PSUM")
            )
            acc_pool = ctx.enter_context(tc.tile_pool(name="acc", bufs=1))

            # paired operand matrices are host-assembled; two HWDGE queues
            xh = const_pool.tile([K, NH], bf16)
            yh = const_pool.tile([K, M], bf16)
            nc.sync.dma_start(xh[:], xh_d[:])
            nc.scalar.dma_start(yh[:], yh_d[:])

            colacc = acc_pool.tile([P, M], f16)
            nc.vector.memset(colacc[:], BIG)
            rowmin = acc_pool.tile([P, NT], f32)

            for t in range(NT):
                z = z_pool.tile([P, M], f16, tag="z")
                lhsT = xh[:, t * P : (t + 1) * P]
                for g in range(4):  # four PSUM groups of 4 banks each
                    ps = psum_pool.tile([P, 4 * 512], f32, tag="ps")
                    for j in range(4):
                        mm = g * 4 + j
                        nc.tensor.matmul(
                            ps[:, j * 512 : (j + 1) * 512],
                            lhsT,
                            yh[:, mm * 512 : (mm + 1) * 512],
                            start=True,
                            stop=True,
                        )
                    nc.scalar.activation(
                        z[:, g * 2048 : (g + 1) * 2048],
                        ps[:],
                        mybir.ActivationFunctionType.Copy,
                    )
                # running per-column min across n-tiles
                nc.vector.tensor_tensor(
                    colacc[:], colacc[:], z[:], mybir.AluOpType.min
                )
                # per-row min of this n-tile
                if ROWMIN_MODE == "scan":
                    # one running-min scan over both halves; last column is
                    # the full row min
                    sc = z_pool.tile([P, M // 2], f16, tag="scan")
                    nc.vector.tensor_tensor_scan(
                        sc[:],
                        z[:, : M // 2],
                        z[:, M // 2 :],
                        initial=BIG,
                        op0=mybir.AluOpType.min,
                        op1=mybir.AluOpType.min,
                    )
                    # tiny extract on the (slack) scalar engine
                    nc.scalar.copy(rowmin[:, t : t + 1], sc[:, M // 2 - 1 : M // 2])
                else:
                    # fold tree at 2x, small reduce
                    src = z
                    w = M
                    while w > FOLD_STOP:
                        w //= 2
                        nxt = z_pool.tile([P, w], f16, tag=f"fold{w}")
                        nc.vector.tensor_tensor(
                            nxt[:], src[:, :w], src[:, w : 2 * w], mybir.AluOpType.min
                        )
                        src = nxt
                    nc.vector.tensor_reduce(
                        rowmin[:, t : t + 1],
                        src[:],
                        axis=mybir.AxisListType.X,
                        op=mybir.AluOpType.min,
                    )

            nc.sync.dma_start(rowmin_d[:], rowmin[:])

            # --- column fold: min over the 128-partition axis of colacc ---
            ident = const_pool.tile([P, P], f16)
            rowidx = const_pool.tile([P, P], f16)
            colidx = const_pool.tile([P, P], f16)
            nc.gpsimd.iota(
                rowidx[:], [[0, P]], channel_multiplier=1,
                allow_small_or_imprecise_dtypes=True,
            )
            nc.gpsimd.iota(
                colidx[:], [[1, P]], channel_multiplier=0,
                allow_small_or_imprecise_dtypes=True,
            )
            nc.vector.tensor_tensor(
                ident[:], rowidx[:], colidx[:], mybir.AluOpType.is_equal
            )

            colmin = acc_pool.tile([P, M // P], f32)
            for half in range(2):
                tp = psum_pool.tile([P, 4096], f16, tag="ps")
                for k in range(32):
                    nc.tensor.transpose(
                        tp[:, k * P : (k + 1) * P],
                        colacc[:, (half * 32 + k) * P : (half * 32 + k + 1) * P],
                        ident[:],
                    )
                nc.vector.tensor_reduce(
                    colmin[:, half * 32 : (half + 1) * 32],
                    tp.rearrange("p (k q) -> p k q", q=P),
                    axis=mybir.AxisListType.X,
                    op=mybir.AluOpType.min,
                )
            nc.sync.dma_start(colmin_d[:], colmin[:])

    nc.compile()
    return nc


def _get_program():
    global _PROGRAM
    if _PROGRAM is None:
        _PROGRAM = _build_program()
    return _PROGRAM


def _split3(a):
    """Split float64 array into 3 bf16 limbs: a ~= l0 + l1 + l2."""
    l0 = a.astype(ml_dtypes.bfloat16)
    r = a - l0.astype(np.float64)
    l1 = r.astype(ml_dtypes.bfloat16)
    r = r - l1.astype(np.float64)
    l2 = r.astype(ml_dtypes.bfloat16)
    return l0, l1, l2


def _unique_rows(pts, negate_double):
    """12 unique bf16 limb rows for one side.

    pts: [L, 3] f64. Rows: 3 limbs of |p|^2, then 3 limbs of each
    coordinate (scaled by -2 when negate_double).
    """
    sq = (pts * pts).sum(1)
    rows = list(_split3(sq))
    scale = -2.0 if negate_double else 1.0
    for i in range(3):
        rows.extend(_split3(scale * pts[:, i]))
    return np.stack(rows).astype(ml_dtypes.bfloat16)


def _core_input(x, y):
    """Flat per-core input: paired x rows [24, NH] then paired y rows [24, M]."""
    xu = _unique_rows(x, negate_double=True)
    yu = _unique_rows(y, negate_double=False)
    ox = np.ones(x.shape[0], ml_dtypes.bfloat16)
    oy = np.ones(y.shape[0], ml_dtypes.bfloat16)
    xh = np.stack([ox if sx == "ones" else xu[sx] for sx, _ in PAIRS])
    yh = np.stack([oy if sy == "ones" else yu[sy] for _, sy in PAIRS])
    return np.concatenate([xh.ravel(), yh.ravel()])


def kernel(prediction, target):
    global LAST_RESULTS
    from concourse.bass_utils import run_bass_kernel_spmd

    nc = _get_program()

    pred = np.asarray(prediction, np.float64)
    tgt = np.asarray(target, np.float64)

    in_maps = []
    for c in range(N_CORES):
        b, h = divmod(c, 2)
        in_maps.append(
            {"xy": _core_input(pred[b, h * NH : (h + 1) * NH], tgt[b])}
        )

    res = run_bass_kernel_spmd(
        nc, in_maps, core_ids=list(range(N_CORES)), trace=TRACE
    )
    LAST_RESULTS = res

    cham_x = np.zeros(B)
    cham_y = np.zeros(B)
    for b in range(B):
        row = []
        cols = []
        for h in range(2):
            r = res.results[2 * b + h]
            # rowmin[p, t] is the min for row n = t*128 + p
            row.append(np.asarray(r["rowmin"], np.float64).T.ravel())
            # colmin[q, k] is the partial min for column m = k*128 + q
            cols.append(np.asarray(r["colmin"], np.float64).T.ravel())
        rowmin = np.concatenate(row)            # [N]
        colmin = np.minimum(cols[0], cols[1])   # [M]
        cham_x[b] = np.maximum(rowmin, 0.0).mean()
        cham_y[b] = np.maximum(colmin, 0.0).mean()

    return np.float32(cham_x.mean() + cham_y.mean())


# revision 40
# speedup vs baseline: 1.0060x; 1.0060x over previous
"""Chamfer distance loss on 8 Trainium2 NeuronCores.

Problem: prediction [4, 8192, 3], target [4, 8192, 3] (f32).
  d2[b,n,m] = ||pred[b,n] - tgt[b,m]||^2  (clamped at 0)
  out = mean_{b,n} min_m d2  +  mean_{b,m} min_n d2     (scalar f32)

Sharding: 8 cores = 4 batches x 2 halves of the N axis. Each core computes
its 4096 x 8192 block of the distance matrix flash-style (never
materialized in DRAM):

  - d2 tiles are produced by a single K=24 bf16 matmul per [128,512] tile
    using the augmented-vector trick  d2 = 1*y2 + x2*1 + sum_i (-2 x_i)*y_i
    with every f32 factor split into 3 bf16 limbs (hi/mid/lo) so products
    are exact to ~2^-26 while the PE runs at full bf16 speed (fp32 matmul
    is 4x slower on the PE).
  - ScalarE drains PSUM -> SBUF converting to f16 (values, not operands,
    so rounding is relative: ~2^-11).
  - VectorE keeps a running per-column min (elementwise f16 min, 2x mode)
    and computes each row min with one tensor_tensor_scan(min,min) over the
    two row halves (the last scan column is the full row min). The fused
    reduce ops tensor_tensor_reduce / tensor_scalar+accum_out crash the HW
    exec unit in this environment; the scan and plain tensor_tensor /
    tensor_reduce are HW-verified here.
  - Column mins are folded over the partition axis on-device (PE
    transpose + DVE reduce) so outputs are tiny (axon tunnel is slow).

The paired 24-row operand matrices are host-assembled and shipped as one
flat bf16 array per core (the axon tunnel charges ~100ms per staged
array, so fewer/larger arrays win).

Host combines: per-batch row mins (exact concat) + per-column partial
mins (elementwise min of the two half-N cores), relu, means.
"""

import sys

if "/opt/trn_rl_repo" not in sys.path:
    sys.path.insert(0, "/opt/trn_rl_repo")

import numpy as np
import ml_dtypes


def _install_neff_cache():
    """Cache compiled NEFFs on disk keyed by BIR hash.

    The stock bass_exec path recompiles (~100s of walrus) in every fresh
    process; the program here is deterministic, so a byte-keyed cache is
    safe and makes repeat runs start in seconds.
    """
    import hashlib
    import os
    import shutil

    from concourse import bass2jax as _b2j
    from concourse import bass_utils as _bu

    if getattr(_bu, "_chamfer_neff_cache", False):
        return
    orig = _bu.compile_bir_kernel

    def _key(bir_json):
        # ant_debug blocks embed the kernel.py path/tracebacks, which vary
        # with the directory the module is loaded from — strip before hashing
        try:
            import orjson

            drop = ("ant_debug", "debug_table")

            def strip(o):
                if isinstance(o, dict):
                    return {k: strip(v) for k, v in o.items() if k not in drop}
                if isinstance(o, list):
                    return [strip(v) for v in o]
                return o

            norm = orjson.dumps(strip(orjson.loads(bir_json)))
        except Exception:
            norm = bir_json
        return hashlib.sha256(norm).hexdigest()

    def cached(bir_json, tmpdir, neff_name="file.neff"):
        key = _key(bir_json)
        cdir = os.environ.get("CHAMFER_NEFF_CACHE", "/tmp/chamfer_neff_cache")
        cpath = os.path.join(cdir, key + ".neff")
        out = os.path.join(tmpdir, neff_name)
        try:
            if os.path.exists(cpath):
                shutil.copyfile(cpath, out)
                return out
        except OSError:
            pass
        p = orig(bir_json, tmpdir, neff_name)
        try:
            os.makedirs(cdir, exist_ok=True)
            tmp = cpath + f".tmp{os.getpid()}"
            shutil.copyfile(p, tmp)
            os.replace(tmp, cpath)
        except OSError:
            pass
        return p

    _bu.compile_bir_kernel = cached
    _b2j.compile_bir_kernel = cached
    _bu._chamfer_neff_cache = True


_install_neff_cache()

B, N, M, D = 4, 8192, 8192, 3
N_CORES = 8
NH = N // 2          # rows per core (4096)
P = 128              # partitions
NT = NH // P         # n-tiles per core (32)
K = 24               # contraction rows of the split-bf16 augmented matmul
BIG = 60000.0        # > max possible d2 (~80), fits in f16
XY_LEN = K * NH + K * M  # paired x rows then paired y rows, flat

import os as _os

Z_BUFS = int(_os.environ.get("CHAMFER_Z_BUFS", "2"))
FOLD_STOP = int(_os.environ.get("CHAMFER_FOLD_STOP", "128"))
ROWMIN_MODE = _os.environ.get("CHAMFER_ROWMIN", "scan")  # fold | scan

# Pairing of the 24 product rows: (x source, y source) where sources index
# the 12 "unique" limb rows per side, or "ones" for the constant row.
# x-unique rows: [x2_0, x2_1, x2_2, a00,a01,a02, a10,a11,a12, a20,a21,a22]
# y-unique rows: [y2_0, y2_1, y2_2, b00,b01,b02, b10,b11,b12, b20,b21,b22]
# where a_i* = limbs of -2*x_i and b_i* = limbs of y_i.
PAIRS = (
    [("ones", 0), ("ones", 1), ("ones", 2), (0, "ones"), (1, "ones"), (2, "ones")]
    + [
        (3 + 3 * i + dx, 3 + 3 * i + dy)
        for i in range(3)
        for dx, dy in ((0, 0), (0, 1), (1, 0), (0, 2), (2, 0), (1, 1))
    ]
)
assert len(PAIRS) == K

# Set by test.py.
TRACE = False
LAST_RESULTS = None

_PROGRAM = None


def _build_program():
    from concourse import bacc, tile
    import concourse.mybir as mybir

    f32 = mybir.dt.float32
    f16 = mybir.dt.float16
    bf16 = mybir.dt.bfloat16

    nc = bacc.Bacc(
        "TRN2",
        target_bir_lowering=False,
        debug=False,
        enable_asserts=False,
    )

    xy_d = nc.dram_tensor("xy", [XY_LEN], bf16, kind="ExternalInput").ap()
    rowmin_d = nc.dram_tensor("rowmin", [P, NT], f32, kind="ExternalOutput").ap()
    # colmin[q, k] = min_p over partitions of column m = 128*k + q
    colmin_d = nc.dram_tensor("colmin", [P, M // P], f32, kind="ExternalOutput").ap()

    xh_d = xy_d[0 : K * NH].rearrange("(k n) -> k n", k=K)
    yh_d = xy_d[K * NH :].rearrange("(k n) -> k n", k=K)

    with tile.TileContext(nc) as tc:
        from contextlib import ExitStack

        with ExitStack() as ctx:
            const_pool = ctx.enter_context(tc.tile_pool(name="const", bufs=1))
            z_pool = ctx.enter_context(tc.tile_pool(name="z", bufs=Z_BUFS))
            psum_pool = ctx.enter_context(
                tc.tile_pool(name="psum", bufs=2, space="PSUM")
            )
            acc_pool = ctx.enter_context(tc.tile_pool(name="acc", bufs=1))

            # paired operand matrices are host-assembled; two HWDGE queues
            xh = const_pool.tile([K, NH], bf16)
            yh = const_pool.tile([K, M], bf16)
            nc.sync.dma_start(xh[:], xh_d[:])
            nc.scalar.dma_start(yh[:, : M // 2], yh_d[:, : M // 2])
            nc.sync.dma_start(yh[:, M // 2 :], yh_d[:, M // 2 :])

            colacc = acc_pool.tile([P, M], f16)
            rowmin = acc_pool.tile([P, NT], f32)

            for t in range(NT):
                z = z_pool.tile([P, M], f16, tag="z")
                lhsT = xh[:, t * P : (t + 1) * P]
                for g in range(4):  # four PSUM groups of 4 banks each
                    ps = psum_pool.tile([P, 4 * 512], f32, tag="ps")
                    for j in range(4):
                        mm = g * 4 + j
                        nc.tensor.matmul(
                            ps[:, j * 512 : (j + 1) * 512],
                            lhsT,
                            yh[:, mm * 512 : (mm + 1) * 512],
                            start=True,
                            stop=True,
                        )
                    nc.scalar.activation(
                        z[:, g * 2048 : (g + 1) * 2048],
                        ps[:],
                        mybir.ActivationFunctionType.Copy,
                    )
                # running per-column min across n-tiles; the first tile just
                # initializes colacc (single-src f16 copy runs at 4x vs the
                # 2x tensor_tensor, and saves the memset)
                if t == 0:
                    nc.vector.tensor_copy(colacc[:], z[:])
                else:
                    nc.vector.tensor_tensor(
                        colacc[:], colacc[:], z[:], mybir.AluOpType.min
                    )
                # per-row min of this n-tile
                if ROWMIN_MODE == "scan":
                    # one running-min scan over both halves; last column is
                    # the full row min
                    sc = z_pool.tile([P, M // 2], f16, tag="scan")
                    nc.vector.tensor_tensor_scan(
                        sc[:],
                        z[:, : M // 2],
                        z[:, M // 2 :],
                        initial=BIG,
                        op0=mybir.AluOpType.min,
                        op1=mybir.AluOpType.min,
                    )
                    # tiny extract on the (slack) scalar engine
                    nc.scalar.copy(rowmin[:, t : t + 1], sc[:, M // 2 - 1 : M // 2])
                else:
                    # fold tree at 2x, small reduce
                    src = z
                    w = M
                    while w > FOLD_STOP:
                        w //= 2
                        nxt = z_pool.tile([P, w], f16, tag=f"fold{w}")
                        nc.vector.tensor_tensor(
                            nxt[:], src[:, :w], src[:, w : 2 * w], mybir.AluOpType.min
                        )
                        src = nxt
                    nc.vector.tensor_reduce(
                        rowmin[:, t : t + 1],
                        src[:],
                        axis=mybir.AxisListType.X,
                        op=mybir.AluOpType.min,
                    )

            nc.sync.dma_start(rowmin_d[:], rowmin[:])

            # --- column fold: min over the 128-partition axis of colacc ---
            ident = const_pool.tile([P, P], f16)
            rowidx = const_pool.tile([P, P], f16)
            colidx = const_pool.tile([P, P], f16)
            nc.gpsimd.iota(
                rowidx[:], [[0, P]], channel_multiplier=1,
                allow_small_or_imprecise_dtypes=True,
            )
            nc.gpsimd.iota(
                colidx[:], [[1, P]], channel_multiplier=0,
                allow_small_or_imprecise_dtypes=True,
            )
            nc.vector.tensor_tensor(
                ident[:], rowidx[:], colidx[:], mybir.AluOpType.is_equal
            )

            colmin = acc_pool.tile([P, M // P], f32)
            for half in range(2):
                tp = psum_pool.tile([P, 4096], f16, tag="ps")
                for k in range(32):
                    nc.tensor.transpose(
                        tp[:, k * P : (k + 1) * P],
                        colacc[:, (half * 32 + k) * P : (half * 32 + k + 1) * P],
                        ident[:],
                    )
                nc.vector.tensor_reduce(
                    colmin[:, half * 32 : (half + 1) * 32],
                    tp.rearrange("p (k q) -> p k q", q=P),
                    axis=mybir.AxisListType.X,
                    op=mybir.AluOpType.min,
                )
            nc.sync.dma_start(colmin_d[:], colmin[:])

    nc.compile()
    return nc


def _get_program():
    global _PROGRAM
    if _PROGRAM is None:
        _PROGRAM = _build_program()
    return _PROGRAM


def _split3(a):
    """Split float64 array into 3 bf16 limbs: a ~= l0 + l1 + l2."""
    l0 = a.astype(ml_dtypes.bfloat16)
    r = a - l0.astype(np.float64)
    l1 = r.astype(ml_dtypes.bfloat16)
    r = r - l1.astype(np.float64)
    l2 = r.astype(ml_dtypes.bfloat16)
    return l0, l1, l2


def _unique_rows(pts, negate_double):
    """12 unique bf16 limb rows for one side.

    pts: [L, 3] f64. Rows: 3 limbs of |p|^2, then 3 limbs of each
    coordinate (scaled by -2 when negate_double).
    """
    sq = (pts * pts).sum(1)
    rows = list(_split3(sq))
    scale = -2.0 if negate_double else 1.0
    for i in range(3):
        rows.extend(_split3(scale * pts[:, i]))
    return np.stack(rows).astype(ml_dtypes.bfloat16)


def _core_input(x, y):
    """Flat per-core input: paired x rows [24, NH] then paired y rows [24, M]."""
    xu = _unique_rows(x, negate_double=True)
    yu = _unique_rows(y, negate_double=False)
    ox = np.ones(x.shape[0], ml_dtypes.bfloat16)
    oy = np.ones(y.shape[0], ml_dtypes.bfloat16)
    xh = np.stack([ox if sx == "ones" else xu[sx] for sx, _ in PAIRS])
    yh = np.stack([oy if sy == "ones" else yu[sy] for _, sy in PAIRS])
    return np.concatenate([xh.ravel(), yh.ravel()])


def kernel(prediction, target):
    global LAST_RESULTS
    from concourse.bass_utils import run_bass_kernel_spmd

    nc = _get_program()

    pred = np.asarray(prediction, np.float64)
    tgt = np.asarray(target, np.float64)

    in_maps = []
    for c in range(N_CORES):
        b, h = divmod(c, 2)
        in_maps.append(
            {"xy": _core_input(pred[b, h * NH : (h + 1) * NH], tgt[b])}
        )

    res = run_bass_kernel_spmd(
        nc, in_maps, core_ids=list(range(N_CORES)), trace=TRACE
    )
    LAST_RESULTS = res

    cham_x = np.zeros(B)
    cham_y = np.zeros(B)
    for b in range(B):
        row = []
        cols = []
        for h in range(2):
            r = res.results[2 * b + h]
            # rowmin[p, t] is the min for row n = t*128 + p
            row.append(np.asarray(r["rowmin"], np.float64).T.ravel())
            # colmin[q, k] is the partial min for column m = k*128 + q
            cols.append(np.asarray(r["colmin"], np.float64).T.ravel())
        rowmin = np.concatenate(row)            # [N]
        colmin = np.minimum(cols[0], cols[1])   # [M]
        cham_x[b] = np.maximum(rowmin, 0.0).mean()
        cham_y[b] = np.maximum(colmin, 0.0).mean()

    return np.float32(cham_x.mean() + cham_y.mean())
